# revision 1
# baseline (speedup 1.0000x reference)
"""Trainium2 Bass kernel for nn_DotProductAttention_292057776923.

Per-head windowed attention with valid-length masking:
  out[h] = softmax(Q[h] K[h]^T / sqrt(d) + wmask[w(h)], masked k>=len[h]) @ V[h]
n=256 heads (B2 x W16 x H8), S=512, d=128, f32.

Sharding: pure head-parallel across 8 cores (32 consecutive heads each);
core c needs window masks [4*(c%4), 4*(c%4)+4). No cross-core communication.

Device algorithm (per head, scoresT layout [k, q] so attention never needs
an on-chip transpose of the [512,512] score matrix):
  - PE-transpose Q,K chunks -> QT,KT [d, S] (f32r-rounded on the PSUM->SBUF cast)
  - scoresT[k_tile] = KT_chunk.T @ QT          (f32r matmul, N=512, full rate)
  - E = Exp(scoresT * scale[k] + bias[k])      (ACT; per-partition scale/bias
        implement /sqrt(d) and valid-len replacement with -60)
  - eT = E * exp(wmask)^T                      (GPSIMD; window mask folded in
        exp-domain; exp(wmask)^T built once per window on-device)
  - out_unnorm[q,:128] | sums[q] = eT_chunk.T @ [V | ones | 0pad]  (f32r, N=256)
  - out = out_unnorm * (1/sums)                (DVE reciprocal + ACT scale-copy)

Valid-length truncation: only ceil(len/128) k-tiles contribute (masked tiles
exponentiate to exp(-60) ~ 1e-26 — exactly the reference's zero weights).
Heads are sorted within each 8-head window group by needed tiles and the
SPMD program uses the per-slot max across cores, so one program serves all
8 cores with ~zero waste. len==0 heads (reference: uniform attention) are
overwritten on the host with mean(V) (~0.5 heads expected per run).
"""
import os
import sys

sys.path.insert(0, "/opt/trn_rl_repo")

import numpy as np
from contextlib import ExitStack

import concourse.bass as bass
import concourse.tile as tile
from concourse import bacc, mybir
from concourse.bass_utils import run_bass_kernel_spmd

F32 = mybir.dt.float32
F32R = mybir.dt.float32r
EXP = mybir.ActivationFunctionType.Exp

N, S, D = 256, 512, 128
NT = S // 128            # 4 k/q tiles per head
N_CORES = 8
HPC = N // N_CORES       # 32 heads per core
WPC = 4                  # window groups per core
HPW = HPC // WPC         # 8 heads per window group
MASK_BIAS = -60.0

USE_F32R = os.environ.get("ATTN_F32R", "1") == "1"
TRUNC = os.environ.get("ATTN_TRUNC", "1") == "1"
ET_BF16 = os.environ.get("ATTN_ET_BF16", "0") == "1"
AVN = 256 if USE_F32R else 132
MMDT = F32R if USE_F32R else F32
ETDT = mybir.dt.bfloat16 if ET_BF16 else MMDT
AVDT = ETDT


def _plan(valid_lens):
    """slot_kt[w][i]: k-tiles computed by slot i of window group w (uniform
    across cores); perm[c][s]: head index (within core) assigned to slot s."""
    kt_head = np.maximum(1, np.ceil(valid_lens / 128.0).astype(np.int64))
    if not TRUNC:
        kt_head[:] = NT
    kt_head = kt_head.reshape(N_CORES, WPC, HPW)
    order = np.argsort(-kt_head, axis=2, kind="stable")      # [C, W, 8]
    sorted_kt = np.take_along_axis(kt_head, order, axis=2)   # [C, W, 8]
    slot_kt = sorted_kt.max(axis=0)                          # [W, 8]
    perm = (order + (np.arange(WPC) * HPW)[None, :, None]).reshape(N_CORES, HPC)
    return slot_kt, perm


def _build_program(slot_kt):
    nc = bacc.Bacc("TRN2", target_bir_lowering=False, debug=False,
                   enable_asserts=True, num_devices=N_CORES)
    q_ap = nc.dram_tensor("q", [HPC, S, D], F32, kind="ExternalInput").ap()
    k_ap = nc.dram_tensor("k", [HPC, S, D], F32, kind="ExternalInput").ap()
    v_ap = nc.dram_tensor("v", [HPC, S, D], AVDT, kind="ExternalInput").ap()
    wm_ap = nc.dram_tensor("wm", [WPC, S, S], F32, kind="ExternalInput").ap()
    id_ap = nc.dram_tensor("ident", [128, 128], F32, kind="ExternalInput").ap()
    op_ap = nc.dram_tensor("onespad", [128, 128], AVDT, kind="ExternalInput").ap()
    sc_ap = nc.dram_tensor("scalev", [128, HPC * NT], F32, kind="ExternalInput").ap()
    bi_ap = nc.dram_tensor("biasv", [128, HPC * NT], F32, kind="ExternalInput").ap()
    o_ap = nc.dram_tensor("o", [HPC, S, D], F32, kind="ExternalOutput").ap()

    with tile.TileContext(nc) as tc, ExitStack() as ctx:
        const_p = ctx.enter_context(tc.tile_pool(name="const", bufs=1))
        qkn = ctx.enter_context(tc.tile_pool(name="qkn", bufs=3))
        qkT = ctx.enter_context(tc.tile_pool(name="qkT", bufs=3))
        vxp = ctx.enter_context(tc.tile_pool(name="vxp", bufs=3))
        wmp = ctx.enter_context(tc.tile_pool(name="wmp", bufs=3))
        ewmp = ctx.enter_context(tc.tile_pool(name="ewmp", bufs=8))
        ep = ctx.enter_context(tc.tile_pool(name="ep", bufs=3))
        etp = ctx.enter_context(tc.tile_pool(name="etp", bufs=4))
        obp = ctx.enter_context(tc.tile_pool(name="obp", bufs=3))
        rp = ctx.enter_context(tc.tile_pool(name="rp", bufs=8))
        pt = ctx.enter_context(tc.tile_pool(name="pt", bufs=2, space="PSUM"))
        ps = ctx.enter_context(tc.tile_pool(name="ps", bufs=2, space="PSUM"))
        po = ctx.enter_context(tc.tile_pool(name="po", bufs=2, space="PSUM"))

        ident = const_p.tile([128, 128], F32)
        nc.sync.dma_start(ident[:], id_ap[:])
        onespad = const_p.tile([128, 128], AVDT)
        nc.sync.dma_start(onespad[:], op_ap[:])
        scv = const_p.tile([128, HPC * NT], F32)
        nc.sync.dma_start(scv[:], sc_ap[:])
        biv = const_p.tile([128, HPC * NT], F32)
        nc.sync.dma_start(biv[:], bi_ap[:])

        # prefetch the first two heads' q/k before window-0 mask prep so
        # PE transposes have operands during the mask pipeline warmup
        prefetched = {}
        for s0 in (0, 1):
            kth0 = int(slot_kt[0][s0])
            qn0 = qkn.tile([128, S], F32, name="qn", tag="qn")
            nc.sync.dma_start(qn0[:], q_ap[s0].rearrange("(t p) d -> p t d", p=128))
            kn0 = qkn.tile([128, S], F32, name="kn", tag="kn")
            nc.sync.dma_start(
                kn0[:, 0:kth0*128],
                k_ap[s0, 0:kth0*128, :].rearrange("(t p) d -> p t d", p=128))
            prefetched[s0] = (qn0, kn0)

        for w in range(WPC):
            ktw = int(slot_kt[w].max())
            # ewm[kt] = exp(wmask[w])^T tiles [k=128, q=512], kt < ktw
            ewm = [ewmp.tile([128, S], F32, name="ewm", tag="ewm")
                   for _ in range(ktw)]
            for qt in range(NT):
                wmn = wmp.tile([128, S], F32, name="wmn", tag="wmn")
                nc.sync.dma_start(wmn[:, 0:ktw*128],
                                  wm_ap[w, qt*128:(qt+1)*128, 0:ktw*128])
                e_nat = wmp.tile([128, S], F32, name="e_nat", tag="e_nat")
                nc.scalar.activation(e_nat[:, 0:ktw*128], wmn[:, 0:ktw*128], EXP)
                ptw = pt.tile([128, 512], F32, name="ptw", tag="ptw")
                for kt in range(ktw):
                    nc.tensor.transpose(ptw[:, kt*128:(kt+1)*128],
                                        e_nat[:, kt*128:(kt+1)*128], ident[:])
                for kt in range(ktw):
                    nc.vector.tensor_copy(ewm[kt][:, qt*128:(qt+1)*128],
                                          ptw[:, kt*128:(kt+1)*128])

            for i in range(HPW):
                s = w * HPW + i
                kth = int(slot_kt[w][i])

                if s in prefetched:
                    qn, kn = prefetched[s]
                else:
                    qn = qkn.tile([128, S], F32, name="qn", tag="qn")
                    nc.sync.dma_start(qn[:], q_ap[s].rearrange("(t p) d -> p t d", p=128))
                    kn = qkn.tile([128, S], F32, name="kn", tag="kn")
                    nc.sync.dma_start(
                        kn[:, 0:kth*128],
                        k_ap[s, 0:kth*128, :].rearrange("(t p) d -> p t d", p=128))

                QT = qkT.tile([128, S], MMDT, name="QT", tag="QT")
                ptq = pt.tile([128, 512], F32, name="ptq", tag="ptw")
                for t in range(NT):
                    nc.tensor.transpose(ptq[:, t*128:(t+1)*128],
                                        qn[:, t*128:(t+1)*128], ident[:])
                nc.vector.tensor_copy(QT[:], ptq[:])

                KT = qkT.tile([128, S], MMDT, name="KT", tag="KT")
                ptk = pt.tile([128, 512], F32, name="ptk", tag="ptw")
                for t in range(kth):
                    nc.tensor.transpose(ptk[:, t*128:(t+1)*128],
                                        kn[:, t*128:(t+1)*128], ident[:])
                nc.vector.tensor_copy(KT[:, 0:kth*128], ptk[:, 0:kth*128])

                # V tiles: [128, kt, AVN] rows of [V | ones | 0pad]
                vxq = vxp.tile([128, NT * AVN], AVDT, name="vxq", tag="vxq")
                vq = vxq.rearrange("p (t n) -> p t n", n=AVN)
                nc.sync.dma_start(
                    vq[:, 0:kth, 0:128],
                    v_ap[s, 0:kth*128, :].rearrange("(t p) d -> p t d", p=128))
                nc.vector.tensor_copy(
                    vq[:, 0:kth, 128:AVN],
                    onespad[:, 0:AVN-128].unsqueeze(1).broadcast_to(
                        [128, kth, AVN - 128]))

                # all 4 q-tiles' accumulators in one 2-bank PSUM tile;
                # start=True only on the first matmul touching each bank's
                # zero region, stop=True only on the last one.
                pov = po.tile([128, NT * AVN], F32, name="pov", tag="pov")
                for kt in range(kth):
                    ps_t = ps.tile([128, S], F32, name="ps_t", tag="ps_t")
                    nc.tensor.matmul(ps_t[:], KT[:, kt*128:(kt+1)*128], QT[:],
                                     start=True, stop=True)
                    E_t = ep.tile([128, S], F32, name="E_t", tag="E_t")
                    c = s * NT + kt
                    nc.scalar.activation(E_t[:], ps_t[:], EXP,
                                         bias=biv[:, c:c+1], scale=scv[:, c:c+1])
                    eT = etp.tile([128, S], ETDT, name="eT", tag="eT")
                    nc.gpsimd.tensor_mul(eT[:, 0:320], E_t[:, 0:320],
                                         ewm[kt][:, 0:320])
                    nc.vector.tensor_mul(eT[:, 320:512], E_t[:, 320:512],
                                         ewm[kt][:, 320:512])
                    for qt in range(NT):
                        nc.tensor.matmul(pov[:, qt*AVN:(qt+1)*AVN],
                                         eT[:, qt*128:(qt+1)*128],
                                         vq[:, kt, :],
                                         start=(kt == 0 and qt % 2 == 0),
                                         stop=(kt == kth-1 and qt % 2 == 1))
                povv = pov.rearrange("p (t n) -> p t n", n=AVN)
                r_t = rp.tile([128, NT], F32, name="r_t", tag="r_t")
                nc.vector.reciprocal(r_t[:], povv[:, :, 128])
                ob = obp.tile([128, S], F32, name="ob", tag="ob")
                for qt in range(NT):
                    nc.scalar.mul(ob[:, qt*128:(qt+1)*128],
                                  povv[:, qt, 0:128], r_t[:, qt:qt+1])
                nc.sync.dma_start(
                    o_ap[s].rearrange("(t p) d -> p t d", p=128), ob[:])
    nc.compile()
    return nc


def _make_in_maps(queries, keys, values, valid_lens, window_mask, perm):
    import ml_dtypes
    av_np_dt = ml_dtypes.bfloat16 if ET_BF16 else np.float32
    isd = 1.0 / np.sqrt(np.float32(D))
    ident_np = np.eye(128, dtype=np.float32)
    onespad_np = np.zeros((128, 128), av_np_dt)
    onespad_np[:, 0] = 1.0

    in_maps = []
    for c in range(N_CORES):
        h0 = c * HPC
        hsel = h0 + perm[c]                              # head for each slot
        lens = valid_lens[hsel]
        kg = np.arange(S)
        valid = kg[None, :] < lens[:, None]              # [HPC(slots), S]
        scalev = np.where(valid, isd, 0.0).astype(np.float32)
        biasv = np.where(valid, 0.0, MASK_BIAS).astype(np.float32)
        scalev = scalev.reshape(HPC, NT, 128).transpose(2, 0, 1).reshape(128, HPC * NT)
        biasv = biasv.reshape(HPC, NT, 128).transpose(2, 0, 1).reshape(128, HPC * NT)
        in_maps.append({
            "q": np.ascontiguousarray(queries[hsel]),
            "k": np.ascontiguousarray(keys[hsel]),
            "v": np.ascontiguousarray(values[hsel].astype(av_np_dt)),
            "wm": np.ascontiguousarray(window_mask[4 * (c % 4): 4 * (c % 4) + 4]),
            "ident": ident_np,
            "onespad": onespad_np,
            "scalev": np.ascontiguousarray(scalev),
            "biasv": np.ascontiguousarray(biasv),
        })
    return in_maps


def _install_ntff_hook():
    import types
    if "antenv.axon_hooks" in sys.modules:
        return
    try:
        from trn_agent_boot.trn_boot import _ntff_profile_via_ctypes
        hook = _ntff_profile_via_ctypes('/opt/axon/libaxon_pjrt.so')
    except Exception:
        hook = None
    mod = types.ModuleType("antenv.axon_hooks")
    mod.get_axon_ntff_profile_hook = lambda: hook
    mod.set_axon_ntff_profile_hook = lambda h: None
    sys.modules["antenv.axon_hooks"] = mod
    try:
        import antenv
        antenv.axon_hooks = mod
    except Exception:
        pass


_LAST_RESULTS = {}


def kernel(queries, keys, values, valid_lens, window_mask):
    queries = np.ascontiguousarray(np.asarray(queries, dtype=np.float32))
    keys = np.ascontiguousarray(np.asarray(keys, dtype=np.float32))
    values = np.ascontiguousarray(np.asarray(values, dtype=np.float32))
    valid_lens = np.asarray(valid_lens, dtype=np.int32)
    window_mask = np.ascontiguousarray(np.asarray(window_mask, dtype=np.float32))

    slot_kt, perm = _plan(valid_lens)
    in_maps = _make_in_maps(queries, keys, values, valid_lens, window_mask, perm)
    nc = _build_program(slot_kt)

    trace = os.environ.get("ATTN_TRACE", "0") == "1"
    if trace:
        _install_ntff_hook()
    res = run_bass_kernel_spmd(nc, in_maps, list(range(N_CORES)), trace=trace)
    _LAST_RESULTS["res"] = res

    out = np.empty((N, S, D), np.float32)
    for c in range(N_CORES):
        out[c * HPC + perm[c]] = res.results[c]["o"]

    # len==0 heads: reference softmaxes an all-(-1e6) row -> uniform
    # attention -> mean of V; the device path can't represent that (the
    # window-mask factor survives exp(-60)). ~0.5 heads expected per run.
    for h in np.nonzero(valid_lens == 0)[0]:
        out[int(h)] = values[int(h)].mean(axis=0, keepdims=True)
    return out



# revision 2
# speedup vs baseline: 1.3529x; 1.3529x over previous
"""Trainium2 Bass kernel for nn_DotProductAttention_292057776923.

Per-head windowed attention with valid-length masking:
  out[h] = softmax(Q[h] K[h]^T / sqrt(d) + wmask[w(h)], masked k>=len[h]) @ V[h]
n=256 heads (B2 x W16 x H8), S=512, d=128, f32.

Sharding: pure head-parallel across 8 cores (32 consecutive heads each);
core c needs window masks [4*(c%4), 4*(c%4)+4). No cross-core communication.

v2 design (per head, scoresT layout [k, q] everywhere):
  - HOST pre-transposes Q, K -> [d, S] (fp16) and wmask -> sqrt(d)*wmask^T
    [k, q] (f32r), and packs V' = [V | ones | pad] (bf16).  No PE transposes,
    no PSUM->SBUF casts, no on-device mask prep.
  - inject:  ps[k,q]  = I.T @ wmT_tile          (PE matmul, start=True)
  - score:   ps[k,q] += KT_chunk.T @ QT          (PE, start=False)
  - exp:     eT[k,q]  = Exp(ps*scale[k]+bias[k]) (ACT; per-partition scale/bias
             implement /sqrt(d) (also un-scaling the sqrt(d)*wmT) and
             valid-len replacement with -60; output bf16)
  - AV+sums: pov[q, qt*256 : qt*256+129] += eT_chunk.T @ [V|ones]   (bf16,
             N=129, accumulated over k-tiles in 2 PSUM banks)
  - ob = one DVE copy of the 4x129 regions -> SBUF; DMA out [128, 516];
    HOST normalizes (out = unnorm / sums) and transposes back.

Valid-length truncation: only ceil(len/128) k-tiles contribute. Heads are
sorted within each 8-head window group by needed tiles and the SPMD program
uses the per-slot max across cores. len==0 heads are overwritten on the host
with mean(V).
"""
import os
import sys

sys.path.insert(0, "/opt/trn_rl_repo")

import numpy as np
from contextlib import ExitStack

import concourse.bass as bass
import concourse.tile as tile
from concourse import bacc, mybir
from concourse.bass_utils import run_bass_kernel_spmd

F32 = mybir.dt.float32
F32R = mybir.dt.float32r
F16 = mybir.dt.float16
BF16 = mybir.dt.bfloat16
EXP = mybir.ActivationFunctionType.Exp

N, S, D = 256, 512, 128
NT = S // 128             # 4 k/q tiles per head
N_CORES = 8
HPC = N // N_CORES        # 32 heads per core
WPC = 4                   # window groups per core
HPW = HPC // WPC          # 8 heads per window group
MASK_BIAS = -60.0
OBW = NT * 129            # 516: per-q-tile [128 outs | 1 sum]

TRUNC = os.environ.get("ATTN_TRUNC", "1") == "1"
QK16 = os.environ.get("ATTN_QK16", "1") == "1"      # Q/K in fp16 (else f32r)
OUT16 = os.environ.get("ATTN_OUT16", "0") == "1"    # ob in bf16 (else f32)
QKDT = F16 if QK16 else F32R
OBDT = BF16 if OUT16 else F32


def _plan(valid_lens):
    """slot_kt[w][i]: k-tiles computed by slot i of window group w (uniform
    across cores); perm[c][s]: head index (within core) assigned to slot s."""
    kt_head = np.maximum(1, np.ceil(valid_lens / 128.0).astype(np.int64))
    if not TRUNC:
        kt_head[:] = NT
    kt_head = kt_head.reshape(N_CORES, WPC, HPW)
    order = np.argsort(-kt_head, axis=2, kind="stable")      # [C, W, 8]
    sorted_kt = np.take_along_axis(kt_head, order, axis=2)   # [C, W, 8]
    slot_kt = sorted_kt.max(axis=0)                          # [W, 8]
    perm = (order + (np.arange(WPC) * HPW)[None, :, None]).reshape(N_CORES, HPC)
    return slot_kt, perm


def _build_program(slot_kt):
    nc = bacc.Bacc("TRN2", target_bir_lowering=False, debug=False,
                   enable_asserts=True, num_devices=N_CORES)
    q_ap = nc.dram_tensor("qT", [HPC, D, S], QKDT, kind="ExternalInput").ap()
    k_ap = nc.dram_tensor("kT", [HPC, D, S], QKDT, kind="ExternalInput").ap()
    v_ap = nc.dram_tensor("vp", [HPC, S, 132], BF16, kind="ExternalInput").ap()
    wm_ap = nc.dram_tensor("wmT", [WPC, S, S], F32R, kind="ExternalInput").ap()
    id_ap = nc.dram_tensor("ident", [128, 128], F32R, kind="ExternalInput").ap()
    sc_ap = nc.dram_tensor("scalev", [128, HPC * NT], F32, kind="ExternalInput").ap()
    bi_ap = nc.dram_tensor("biasv", [128, HPC * NT], F32, kind="ExternalInput").ap()
    o_ap = nc.dram_tensor("o", [HPC, 128, OBW], OBDT, kind="ExternalOutput").ap()

    with tile.TileContext(nc) as tc, ExitStack() as ctx:
        const_p = ctx.enter_context(tc.tile_pool(name="const", bufs=1))
        qp = ctx.enter_context(tc.tile_pool(name="qp", bufs=3))
        kp = ctx.enter_context(tc.tile_pool(name="kp", bufs=3))
        vp = ctx.enter_context(tc.tile_pool(name="vp", bufs=3))
        wmp = ctx.enter_context(tc.tile_pool(name="wmp", bufs=2))
        etp = ctx.enter_context(tc.tile_pool(name="etp", bufs=6))
        obp = ctx.enter_context(tc.tile_pool(name="obp", bufs=3))
        ps = ctx.enter_context(tc.tile_pool(name="ps", bufs=4, space="PSUM"))
        po = ctx.enter_context(tc.tile_pool(name="po", bufs=2, space="PSUM"))

        ident = const_p.tile([128, 128], F32R)
        nc.sync.dma_start(ident[:], id_ap[:])
        scv = const_p.tile([128, HPC * NT], F32)
        nc.sync.dma_start(scv[:], sc_ap[:])
        biv = const_p.tile([128, HPC * NT], F32)
        nc.sync.dma_start(biv[:], bi_ap[:])

        # prefetch the first two heads' q/k/v before window-0 mask DMA
        prefetched = {}
        for s0 in (0, 1):
            kth0 = int(slot_kt[0][s0])
            qn0 = qp.tile([128, S], QKDT, name="qn", tag="qn")
            nc.sync.dma_start(qn0[:], q_ap[s0])
            kn0 = kp.tile([128, S], QKDT, name="kn", tag="kn")
            nc.sync.dma_start(kn0[:, 0:kth0*128], k_ap[s0, :, 0:kth0*128])
            vn0 = vp.tile([128, NT * 132], BF16, name="vn", tag="vn")
            nc.sync.dma_start(
                vn0.rearrange("p (t c) -> p t c", c=132)[:, 0:kth0, :],
                v_ap[s0, 0:kth0*128, :].rearrange("(t p) c -> p t c", p=128))
            prefetched[s0] = (qn0, kn0, vn0)

        for w in range(WPC):
            ktw = int(slot_kt[w].max())
            # wmt[k=128, (kt, q)] tiles of sqrt(d)*wmask[w]^T for kt < ktw
            wmt = wmp.tile([128, NT * S], F32R, name="wmt", tag="wmt")
            wmtv = wmt.rearrange("p (t q) -> p t q", q=S)
            nc.sync.dma_start(
                wmtv[:, 0:ktw, :],
                wm_ap[w, 0:ktw*128, :].rearrange("(t p) q -> p t q", p=128))

            for i in range(HPW):
                s = w * HPW + i
                kth = int(slot_kt[w][i])

                if s in prefetched:
                    qn, kn, vn = prefetched[s]
                else:
                    qn = qp.tile([128, S], QKDT, name="qn", tag="qn")
                    nc.sync.dma_start(qn[:], q_ap[s])
                    kn = kp.tile([128, S], QKDT, name="kn", tag="kn")
                    nc.sync.dma_start(kn[:, 0:kth*128], k_ap[s, :, 0:kth*128])
                    vn = vp.tile([128, NT * 132], BF16, name="vn", tag="vn")
                    nc.sync.dma_start(
                        vn.rearrange("p (t c) -> p t c", c=132)[:, 0:kth, :],
                        v_ap[s, 0:kth*128, :].rearrange("(t p) c -> p t c", p=128))
                vnv = vn.rearrange("p (t c) -> p t c", c=132)

                # window-mask inject: ps[kt] = sqrt(d)*wmT tile (start=True)
                pst = []
                for kt in range(kth):
                    p_t = ps.tile([128, S], F32, name="ps_t", tag="ps_t")
                    nc.tensor.matmul(p_t[:], ident[:], wmtv[:, kt, :],
                                     start=True, stop=False)
                    pst.append(p_t)

                pov = po.tile([128, NT * 256], F32, name="pov", tag="pov")
                for kt in range(kth):
                    nc.tensor.matmul(pst[kt][:], kn[:, kt*128:(kt+1)*128],
                                     qn[:], start=False, stop=True)
                    eT = etp.tile([128, S], BF16, name="eT", tag="eT")
                    c = s * NT + kt
                    nc.scalar.activation(eT[:], pst[kt][:], EXP,
                                         bias=biv[:, c:c+1], scale=scv[:, c:c+1])
                    for qt in range(NT):
                        nc.tensor.matmul(pov[:, qt*256:qt*256+129],
                                         eT[:, qt*128:(qt+1)*128],
                                         vnv[:, kt, 0:129],
                                         start=(kt == 0 and qt % 2 == 0),
                                         stop=(kt == kth-1 and qt % 2 == 1))

                povv = pov.rearrange("p (t c) -> p t c", c=256)
                ob = obp.tile([128, OBW], OBDT, name="ob", tag="ob")
                obv = ob.rearrange("p (t c) -> p t c", c=129)
                nc.vector.tensor_copy(obv[:], povv[:, :, 0:129])
                nc.sync.dma_start(o_ap[s], ob[:])
    nc.compile()
    return nc


def _make_in_maps(queries, keys, values, valid_lens, window_mask, perm):
    import ml_dtypes
    qk_np_dt = ml_dtypes.bfloat16 if False else (
        np.float16 if QK16 else np.float32)
    isd = 1.0 / np.sqrt(np.float32(D))
    sd = np.sqrt(np.float32(D))
    ident_np = np.eye(128, dtype=np.float32)

    in_maps = []
    for c in range(N_CORES):
        h0 = c * HPC
        hsel = h0 + perm[c]                              # head for each slot
        lens = valid_lens[hsel]
        kg = np.arange(S)
        valid = kg[None, :] < lens[:, None]              # [HPC(slots), S]
        scalev = np.where(valid, isd, 0.0).astype(np.float32)
        biasv = np.where(valid, 0.0, MASK_BIAS).astype(np.float32)
        scalev = scalev.reshape(HPC, NT, 128).transpose(2, 0, 1).reshape(128, HPC * NT)
        biasv = biasv.reshape(HPC, NT, 128).transpose(2, 0, 1).reshape(128, HPC * NT)

        vpk = np.zeros((HPC, S, 132), ml_dtypes.bfloat16)
        vpk[:, :, 0:128] = values[hsel]
        vpk[:, :, 128] = 1.0

        wmT = (sd * window_mask[4 * (c % 4): 4 * (c % 4) + 4]
               ).transpose(0, 2, 1)                      # [4, k, q]

        in_maps.append({
            "qT": np.ascontiguousarray(
                queries[hsel].transpose(0, 2, 1).astype(qk_np_dt)),
            "kT": np.ascontiguousarray(
                keys[hsel].transpose(0, 2, 1).astype(qk_np_dt)),
            "vp": np.ascontiguousarray(vpk),
            "wmT": np.ascontiguousarray(wmT.astype(np.float32)),
            "ident": ident_np,
            "scalev": np.ascontiguousarray(scalev),
            "biasv": np.ascontiguousarray(biasv),
        })
    return in_maps


def _postprocess(ob_core):
    """[HPC, 128, 516] device output -> [HPC, S, D] normalized, q-major."""
    a = np.asarray(ob_core, np.float32).reshape(HPC, 128, NT, 129)
    unnorm = a[:, :, :, 0:128]
    sums = a[:, :, :, 128:129]
    out = unnorm / sums                                  # [HPC, qp, qt, d]
    return out.transpose(0, 2, 1, 3).reshape(HPC, S, D)


def _install_ntff_hook():
    import types
    if "antenv.axon_hooks" in sys.modules:
        return
    try:
        from trn_agent_boot.trn_boot import _ntff_profile_via_ctypes
        hook = _ntff_profile_via_ctypes('/opt/axon/libaxon_pjrt.so')
    except Exception:
        hook = None
    mod = types.ModuleType("antenv.axon_hooks")
    mod.get_axon_ntff_profile_hook = lambda: hook
    mod.set_axon_ntff_profile_hook = lambda h: None
    sys.modules["antenv.axon_hooks"] = mod
    try:
        import antenv
        antenv.axon_hooks = mod
    except Exception:
        pass


_LAST_RESULTS = {}


def kernel(queries, keys, values, valid_lens, window_mask):
    queries = np.ascontiguousarray(np.asarray(queries, dtype=np.float32))
    keys = np.ascontiguousarray(np.asarray(keys, dtype=np.float32))
    values = np.ascontiguousarray(np.asarray(values, dtype=np.float32))
    valid_lens = np.asarray(valid_lens, dtype=np.int32)
    window_mask = np.ascontiguousarray(np.asarray(window_mask, dtype=np.float32))

    slot_kt, perm = _plan(valid_lens)
    in_maps = _make_in_maps(queries, keys, values, valid_lens, window_mask, perm)
    nc = _build_program(slot_kt)

    trace = os.environ.get("ATTN_TRACE", "0") == "1"
    if trace:
        _install_ntff_hook()
    res = run_bass_kernel_spmd(nc, in_maps, list(range(N_CORES)), trace=trace)
    _LAST_RESULTS["res"] = res

    out = np.empty((N, S, D), np.float32)
    for c in range(N_CORES):
        out[c * HPC + perm[c]] = _postprocess(res.results[c]["o"])

    # len==0 heads: reference softmaxes an all-(-1e6) row -> uniform
    # attention -> mean of V; the device path can't represent that.
    for h in np.nonzero(valid_lens == 0)[0]:
        out[int(h)] = values[int(h)].mean(axis=0, keepdims=True)
    return out


# revision 3
# speedup vs baseline: 1.5174x; 1.1216x over previous
"""Trainium2 Bass kernel for nn_DotProductAttention_292057776923.

Per-head windowed attention with valid-length masking:
  out[h] = softmax(Q[h] K[h]^T / sqrt(d) + wmask[w(h)], masked k>=len[h]) @ V[h]
n=256 heads (B2 x W16 x H8), S=512, d=128, f32.

Sharding: pure head-parallel across 8 cores (32 consecutive heads each);
core c needs window masks [4*(c%4), 4*(c%4)+4). No cross-core communication.

v3 design (per head, scoresT layout [k, q] everywhere):
  - HOST pre-transposes Q, K -> one packed [d, 1024] fp16 slab per head,
    wmask -> sqrt(d)*wmask^T (fp16), and packs V' = [V | ones | pad] (bf16).
    No PE transposes, no PSUM->SBUF casts, no on-device mask prep.
  - inject:  ps[k,q]  = I.T @ wmT_tile          (PE matmul, start=True)
  - score:   ps[k,q] += KT_chunk.T @ QT          (PE, start=False)
  - exp:     eT[k,q]  = Exp(ps*scale[k]+bias[k]) (ACT -> bf16; k-tile PAIRS
             that are fully interior on every core run as one [128,1024]
             ACT with immediate scale/bias)
  - AV+sums: pov[q, qt*256 : qt*256+129] += eT_chunk.T @ [V|ones]   (bf16,
             N=129, accumulated over k-tiles in 2 PSUM banks)
  - ob = one DVE copy of the 4x129 regions -> SBUF (bf16); DMA out;
    HOST normalizes (out = unnorm / sums) and transposes back.
  - input DMA triggers ride the (otherwise idle) GpSimd queue; stores on
    Sync - avoids serializing ~100 DMA triggers on one engine.

Valid-length truncation: only ceil(len/128) k-tiles contribute. Heads are
sorted within each 8-head window group by needed tiles and the SPMD program
uses the per-slot max across cores. len==0 heads are overwritten on the host
with mean(V).
"""
import os
import sys

sys.path.insert(0, "/opt/trn_rl_repo")

import numpy as np
from contextlib import ExitStack

import concourse.bass as bass
import concourse.tile as tile
from concourse import bacc, mybir
from concourse.bass_utils import run_bass_kernel_spmd

F32 = mybir.dt.float32
F32R = mybir.dt.float32r
F16 = mybir.dt.float16
BF16 = mybir.dt.bfloat16
EXP = mybir.ActivationFunctionType.Exp

N, S, D = 256, 512, 128
NT = S // 128             # 4 k/q tiles per head
N_CORES = 8
HPC = N // N_CORES        # 32 heads per core
WPC = 4                   # window groups per core
HPW = HPC // WPC          # 8 heads per window group
MASK_BIAS = -60.0
OBW = NT * 129            # 516: per-q-tile [128 outs | 1 sum]
ISD = 1.0 / float(np.sqrt(np.float32(D)))

TRUNC = os.environ.get("ATTN_TRUNC", "1") == "1"
QK16 = os.environ.get("ATTN_QK16", "1") == "1"      # Q/K in fp16 (else f32r)
OUT16 = os.environ.get("ATTN_OUT16", "1") == "1"    # ob in bf16 (else f32)
WM16 = os.environ.get("ATTN_WM16", "1") == "1"      # wmT in fp16 (else f32r)
PAIR = os.environ.get("ATTN_PAIR", "1") == "1"      # merge interior exp pairs
QKDT = F16 if QK16 else F32R
OBDT = BF16 if OUT16 else F32
WMDT = F16 if WM16 else F32R


def _plan(valid_lens):
    """slot_kt[w][i]: k-tiles computed by slot i of window group w (uniform
    across cores); slot_uni[w][i]: #leading k-tiles fully valid on EVERY
    core (safe for immediate-scale merged exp); perm[c][s]: head index
    (within core) assigned to slot s."""
    kt_head = np.maximum(1, np.ceil(valid_lens / 128.0).astype(np.int64))
    if not TRUNC:
        kt_head[:] = NT
    kt_head = kt_head.reshape(N_CORES, WPC, HPW)
    order = np.argsort(-kt_head, axis=2, kind="stable")      # [C, W, 8]
    sorted_kt = np.take_along_axis(kt_head, order, axis=2)   # [C, W, 8]
    slot_kt = sorted_kt.max(axis=0)                          # [W, 8]
    perm = (order + (np.arange(WPC) * HPW)[None, :, None]).reshape(N_CORES, HPC)

    lens = valid_lens.reshape(N_CORES, WPC, HPW)
    sorted_len = np.take_along_axis(lens, order, axis=2)     # [C, W, 8]
    slot_uni = (sorted_len.min(axis=0) // 128)               # [W, 8]
    if not PAIR:
        slot_uni[:] = 0
    return slot_kt, slot_uni, perm


def _build_program(slot_kt, slot_uni):
    nc = bacc.Bacc("TRN2", target_bir_lowering=False, debug=False,
                   enable_asserts=True, num_devices=N_CORES)
    qk_ap = nc.dram_tensor("qk", [HPC, D, 2 * S], QKDT, kind="ExternalInput").ap()
    v_ap = nc.dram_tensor("vp", [HPC, S, 132], BF16, kind="ExternalInput").ap()
    wm_ap = nc.dram_tensor("wmT", [WPC, S, S], WMDT, kind="ExternalInput").ap()
    id_ap = nc.dram_tensor("ident", [128, 128], WMDT, kind="ExternalInput").ap()
    sc_ap = nc.dram_tensor("scalev", [128, HPC * NT], F32, kind="ExternalInput").ap()
    bi_ap = nc.dram_tensor("biasv", [128, HPC * NT], F32, kind="ExternalInput").ap()
    o_ap = nc.dram_tensor("o", [HPC, 128, OBW], OBDT, kind="ExternalOutput").ap()

    with tile.TileContext(nc) as tc, ExitStack() as ctx:
        const_p = ctx.enter_context(tc.tile_pool(name="const", bufs=1))
        qkp = ctx.enter_context(tc.tile_pool(name="qkp", bufs=4))
        vpool = ctx.enter_context(tc.tile_pool(name="vpool", bufs=4))
        wmp = ctx.enter_context(tc.tile_pool(name="wmp", bufs=2))
        etp = ctx.enter_context(tc.tile_pool(name="etp", bufs=3))
        obp = ctx.enter_context(tc.tile_pool(name="obp", bufs=3))
        ps = ctx.enter_context(tc.tile_pool(name="ps", bufs=2, space="PSUM"))
        po = ctx.enter_context(tc.tile_pool(name="po", bufs=2, space="PSUM"))

        ident = const_p.tile([128, 128], WMDT)
        nc.gpsimd.dma_start(ident[:], id_ap[:])
        scv = const_p.tile([128, HPC * NT], F32)
        nc.gpsimd.dma_start(scv[:], sc_ap[:])
        biv = const_p.tile([128, HPC * NT], F32)
        nc.gpsimd.dma_start(biv[:], bi_ap[:])

        # prefetch the first two heads' q/k/v before window-0 mask DMA
        prefetched = {}
        for s0 in (0, 1):
            kth0 = int(slot_kt[0][s0])
            qkn0 = qkp.tile([128, 2 * S], QKDT, name="qkn", tag="qkn")
            nc.gpsimd.dma_start(qkn0[:, 0:S+kth0*128], qk_ap[s0, :, 0:S+kth0*128])
            vn0 = vpool.tile([128, NT * 132], BF16, name="vn", tag="vn")
            nc.gpsimd.dma_start(
                vn0.rearrange("p (t c) -> p t c", c=132)[:, 0:kth0, :],
                v_ap[s0, 0:kth0*128, :].rearrange("(t p) c -> p t c", p=128))
            prefetched[s0] = (qkn0, vn0)

        for w in range(WPC):
            ktw = int(slot_kt[w].max())
            # wmt[k=128, (kt, q)] tiles of sqrt(d)*wmask[w]^T for kt < ktw
            wmt = wmp.tile([128, NT * S], WMDT, name="wmt", tag="wmt")
            wmtv = wmt.rearrange("p (t q) -> p t q", q=S)
            nc.gpsimd.dma_start(
                wmtv[:, 0:ktw, :],
                wm_ap[w, 0:ktw*128, :].rearrange("(t p) q -> p t q", p=128))

            for i in range(HPW):
                s = w * HPW + i
                kth = int(slot_kt[w][i])
                uni = int(slot_uni[w][i])

                if s in prefetched:
                    qkn, vn = prefetched[s]
                else:
                    qkn = qkp.tile([128, 2 * S], QKDT, name="qkn", tag="qkn")
                    nc.gpsimd.dma_start(qkn[:, 0:S+kth*128],
                                        qk_ap[s, :, 0:S+kth*128])
                    vn = vpool.tile([128, NT * 132], BF16, name="vn", tag="vn")
                    nc.gpsimd.dma_start(
                        vn.rearrange("p (t c) -> p t c", c=132)[:, 0:kth, :],
                        v_ap[s, 0:kth*128, :].rearrange("(t p) c -> p t c", p=128))
                vnv = vn.rearrange("p (t c) -> p t c", c=132)

                # k-tile pairs share a 2-bank PSUM tile so interior pairs can
                # run a single merged [128,1024] exp
                npair = (kth + 1) // 2
                pst = [ps.tile([128, 2 * S], F32, name="ps_t", tag="ps_t")
                       for _ in range(npair)]

                def ps_half(kt):
                    return pst[kt // 2][:, (kt % 2)*S:(kt % 2)*S + S]

                for kt in range(kth):
                    nc.tensor.matmul(ps_half(kt), ident[:], wmtv[:, kt, :],
                                     start=True, stop=False)

                pov = po.tile([128, NT * 256], F32, name="pov", tag="pov")
                ets = {}
                for kt in range(kth):
                    nc.tensor.matmul(ps_half(kt), qkn[:, S+kt*128:S+(kt+1)*128],
                                     qkn[:, 0:S], start=False, stop=True)
                    # exp as soon as a pair (or tail) is fully scored
                    if kt % 2 == 1 or kt == kth - 1:
                        p0 = kt - (kt % 2)
                        eT = etp.tile([128, 2 * S], BF16, name="eT", tag="eT")
                        if kt % 2 == 1 and kt < uni:
                            # both k-tiles fully valid on every core
                            nc.scalar.activation(
                                eT[:], pst[kt // 2][:], EXP,
                                bias=0.0, scale=ISD)
                        else:
                            for k2 in range(p0, kt + 1):
                                c = s * NT + k2
                                nc.scalar.activation(
                                    eT[:, (k2 % 2)*S:(k2 % 2)*S + S],
                                    ps_half(k2), EXP,
                                    bias=biv[:, c:c+1], scale=scv[:, c:c+1])
                        for k2 in range(p0, kt + 1):
                            ets[k2] = eT[:, (k2 % 2)*S:(k2 % 2)*S + S]
                            for qt in range(NT):
                                nc.tensor.matmul(
                                    pov[:, qt*256:qt*256+129],
                                    ets[k2][:, qt*128:(qt+1)*128],
                                    vnv[:, k2, 0:129],
                                    start=(k2 == 0 and qt % 2 == 0),
                                    stop=(k2 == kth-1 and qt % 2 == 1))

                povv = pov.rearrange("p (t c) -> p t c", c=256)
                ob = obp.tile([128, OBW], OBDT, name="ob", tag="ob")
                obv = ob.rearrange("p (t c) -> p t c", c=129)
                nc.vector.tensor_copy(obv[:], povv[:, :, 0:129])
                nc.sync.dma_start(o_ap[s], ob[:])
    nc.compile()
    return nc


def _make_in_maps(queries, keys, values, valid_lens, window_mask, perm):
    import ml_dtypes
    qk_np_dt = np.float16 if QK16 else np.float32
    wm_np_dt = np.float16 if WM16 else np.float32
    isd = 1.0 / np.sqrt(np.float32(D))
    sd = np.sqrt(np.float32(D))
    ident_np = np.eye(128, dtype=wm_np_dt)

    in_maps = []
    for c in range(N_CORES):
        h0 = c * HPC
        hsel = h0 + perm[c]                              # head for each slot
        lens = valid_lens[hsel]
        kg = np.arange(S)
        valid = kg[None, :] < lens[:, None]              # [HPC(slots), S]
        scalev = np.where(valid, isd, 0.0).astype(np.float32)
        biasv = np.where(valid, 0.0, MASK_BIAS).astype(np.float32)
        scalev = scalev.reshape(HPC, NT, 128).transpose(2, 0, 1).reshape(128, HPC * NT)
        biasv = biasv.reshape(HPC, NT, 128).transpose(2, 0, 1).reshape(128, HPC * NT)

        qkn = np.empty((HPC, D, 2 * S), qk_np_dt)
        qkn[:, :, 0:S] = queries[hsel].transpose(0, 2, 1)
        qkn[:, :, S:2*S] = keys[hsel].transpose(0, 2, 1)

        vpk = np.zeros((HPC, S, 132), ml_dtypes.bfloat16)
        vpk[:, :, 0:128] = values[hsel]
        vpk[:, :, 128] = 1.0

        wmT = (sd * window_mask[4 * (c % 4): 4 * (c % 4) + 4]
               ).transpose(0, 2, 1)                      # [4, k, q]

        in_maps.append({
            "qk": np.ascontiguousarray(qkn),
            "vp": np.ascontiguousarray(vpk),
            "wmT": np.ascontiguousarray(wmT.astype(wm_np_dt)),
            "ident": ident_np,
            "scalev": np.ascontiguousarray(scalev),
            "biasv": np.ascontiguousarray(biasv),
        })
    return in_maps


def _postprocess(ob_core):
    """[HPC, 128, 516] device output -> [HPC, S, D] normalized, q-major."""
    a = np.asarray(ob_core, np.float32).reshape(HPC, 128, NT, 129)
    unnorm = a[:, :, :, 0:128]
    sums = a[:, :, :, 128:129]
    out = unnorm / sums                                  # [HPC, qp, qt, d]
    return out.transpose(0, 2, 1, 3).reshape(HPC, S, D)


def _install_ntff_hook():
    import types
    if "antenv.axon_hooks" in sys.modules:
        return
    try:
        from trn_agent_boot.trn_boot import _ntff_profile_via_ctypes
        hook = _ntff_profile_via_ctypes('/opt/axon/libaxon_pjrt.so')
    except Exception:
        hook = None
    mod = types.ModuleType("antenv.axon_hooks")
    mod.get_axon_ntff_profile_hook = lambda: hook
    mod.set_axon_ntff_profile_hook = lambda h: None
    sys.modules["antenv.axon_hooks"] = mod
    try:
        import antenv
        antenv.axon_hooks = mod
    except Exception:
        pass


_LAST_RESULTS = {}


def kernel(queries, keys, values, valid_lens, window_mask):
    queries = np.ascontiguousarray(np.asarray(queries, dtype=np.float32))
    keys = np.ascontiguousarray(np.asarray(keys, dtype=np.float32))
    values = np.ascontiguousarray(np.asarray(values, dtype=np.float32))
    valid_lens = np.asarray(valid_lens, dtype=np.int32)
    window_mask = np.ascontiguousarray(np.asarray(window_mask, dtype=np.float32))

    slot_kt, slot_uni, perm = _plan(valid_lens)
    in_maps = _make_in_maps(queries, keys, values, valid_lens, window_mask, perm)
    nc = _build_program(slot_kt, slot_uni)

    trace = os.environ.get("ATTN_TRACE", "0") == "1"
    if trace:
        _install_ntff_hook()
    res = run_bass_kernel_spmd(nc, in_maps, list(range(N_CORES)), trace=trace)
    _LAST_RESULTS["res"] = res

    out = np.empty((N, S, D), np.float32)
    for c in range(N_CORES):
        out[c * HPC + perm[c]] = _postprocess(res.results[c]["o"])

    # len==0 heads: reference softmaxes an all-(-1e6) row -> uniform
    # attention -> mean of V; the device path can't represent that.
    for h in np.nonzero(valid_lens == 0)[0]:
        out[int(h)] = values[int(h)].mean(axis=0, keepdims=True)
    return out


# revision 11
# speedup vs baseline: 1.5849x; 1.0445x over previous
"""Trainium2 Bass kernel for nn_DotProductAttention_292057776923.

Per-head windowed attention with valid-length masking:
  out[h] = softmax(Q[h] K[h]^T / sqrt(d) + wmask[w(h)], masked k>=len[h]) @ V[h]
n=256 heads (B2 x W16 x H8), S=512, d=128, f32.

Sharding: pure head-parallel across 8 cores (32 consecutive heads each);
core c needs window masks [4*(c%4), 4*(c%4)+4). No cross-core communication.

v3 design (per head, scoresT layout [k, q] everywhere):
  - HOST pre-transposes Q, K -> one packed [d, 1024] fp16 slab per head,
    wmask -> sqrt(d)*wmask^T (fp16), and packs V' = [V | ones | pad] (bf16).
    No PE transposes, no PSUM->SBUF casts, no on-device mask prep.
  - inject:  ps[k,q]  = I.T @ wmT_tile          (PE matmul, start=True)
  - score:   ps[k,q] += KT_chunk.T @ QT          (PE, start=False)
  - exp:     eT[k,q]  = Exp(ps*scale[k]+bias[k]) (ACT -> bf16; k-tile PAIRS
             that are fully interior on every core run as one [128,1024]
             ACT with immediate scale/bias)
  - AV+sums: pov[q, qt*256 : qt*256+129] += eT_chunk.T @ [V|ones]   (bf16,
             N=129, accumulated over k-tiles in 2 PSUM banks)
  - ob = one DVE copy of the 4x129 regions -> SBUF (bf16); DMA out;
    HOST normalizes (out = unnorm / sums) and transposes back.
  - input DMA triggers ride the (otherwise idle) GpSimd queue; stores on
    Sync - avoids serializing ~100 DMA triggers on one engine.

Valid-length truncation: only ceil(len/128) k-tiles contribute. Heads are
sorted within each 8-head window group by needed tiles and the SPMD program
uses the per-slot max across cores. len==0 heads are overwritten on the host
with mean(V).
"""
import os
import sys

sys.path.insert(0, "/opt/trn_rl_repo")

import numpy as np
from contextlib import ExitStack

import concourse.bass as bass
import concourse.tile as tile
from concourse import bacc, mybir
from concourse.bass_utils import run_bass_kernel_spmd

F32 = mybir.dt.float32
F32R = mybir.dt.float32r
F16 = mybir.dt.float16
BF16 = mybir.dt.bfloat16
EXP = mybir.ActivationFunctionType.Exp

N, S, D = 256, 512, 128
NT = S // 128             # 4 k/q tiles per head
N_CORES = 8
HPC = N // N_CORES        # 32 heads per core
WPC = 4                   # window groups per core
HPW = HPC // WPC          # 8 heads per window group
MASK_BIAS = -60.0
OBW = NT * 129            # 516: per-q-tile [128 outs | 1 sum]
ISD = 1.0 / float(np.sqrt(np.float32(D)))

TRUNC = os.environ.get("ATTN_TRUNC", "1") == "1"
QK16 = os.environ.get("ATTN_QK16", "1") == "1"      # Q/K in fp16 (else f32r)
OUT16 = os.environ.get("ATTN_OUT16", "1") == "1"    # ob in bf16 (else f32)
WM16 = os.environ.get("ATTN_WM16", "1") == "1"      # wmT in fp16 (else f32r)
PAIR = os.environ.get("ATTN_PAIR", "1") == "1"      # merge interior exp pairs
QKDT = F16 if QK16 else F32R
OBDT = BF16 if OUT16 else F32
WMDT = F16 if WM16 else F32R


def _plan(valid_lens):
    """slot_kt[w][i]: k-tiles computed by slot i of window group w (uniform
    across cores); hsel[c][s]: GLOBAL head index assigned to core c, slot s.

    Cores c and c+4 use the same 4 windows, so each (window, core-pair)
    pool of 16 heads is sorted by k-tiles and snake-split: core c gets
    even ranks, c+4 odd ranks.  Slot i's k-tile count is then the rank-2i
    value, maxed over the 4 pools only."""
    kt_head = np.maximum(1, np.ceil(valid_lens / 128.0).astype(np.int64))
    if not TRUNC:
        kt_head[:] = NT
    hsel = np.empty((N_CORES, HPC), np.int64)
    slot_kt = np.zeros((WPC, HPW), np.int64)
    for p in range(4):                       # core pair (p, p+4)
        for w in range(WPC):
            gw = 4 * p + w                   # global window index
            pool = np.concatenate([
                np.arange(p * HPC + w * HPW, p * HPC + (w + 1) * HPW),
                np.arange((p + 4) * HPC + w * HPW, (p + 4) * HPC + (w + 1) * HPW),
            ])
            order = pool[np.argsort(-kt_head[pool], kind="stable")]
            hsel[p, w * HPW:(w + 1) * HPW] = order[0::2]
            hsel[p + 4, w * HPW:(w + 1) * HPW] = order[1::2]
            slot_kt[w] = np.maximum(slot_kt[w], kt_head[order[0::2]])
    return slot_kt, hsel


def _build_program(slot_kt):
    nc = bacc.Bacc("TRN2", target_bir_lowering=False, debug=False,
                   enable_asserts=True, num_devices=N_CORES)
    qk_ap = nc.dram_tensor("qk", [HPC, D, 2 * S], QKDT, kind="ExternalInput").ap()
    v_ap = nc.dram_tensor("vp", [HPC, S, 132], BF16, kind="ExternalInput").ap()
    wm_ap = nc.dram_tensor("wmT", [WPC, S, S], WMDT, kind="ExternalInput").ap()
    id_ap = nc.dram_tensor("ident", [128, 128], WMDT, kind="ExternalInput").ap()
    o_ap = nc.dram_tensor("o", [HPC, 128, OBW], OBDT, kind="ExternalOutput").ap()

    with tile.TileContext(nc) as tc, ExitStack() as ctx:
        const_p = ctx.enter_context(tc.tile_pool(name="const", bufs=1))
        qkp = ctx.enter_context(tc.tile_pool(name="qkp", bufs=4))
        vpool = ctx.enter_context(tc.tile_pool(name="vpool", bufs=4))
        wmp = ctx.enter_context(tc.tile_pool(name="wmp", bufs=2))
        etp = ctx.enter_context(tc.tile_pool(name="etp", bufs=3))
        obp = ctx.enter_context(tc.tile_pool(name="obp", bufs=3))
        ps = ctx.enter_context(tc.tile_pool(name="ps", bufs=2, space="PSUM"))
        po = ctx.enter_context(tc.tile_pool(name="po", bufs=2, space="PSUM"))

        ident = const_p.tile([128, 128], WMDT)
        nc.gpsimd.dma_start(ident[:], id_ap[:])

        # prefetch the first two heads' q/k/v before window-0 mask DMA
        prefetched = {}
        for s0 in (0, 1):
            kth0 = int(slot_kt[0][s0])
            qkn0 = qkp.tile([128, 2 * S], QKDT, name="qkn", tag="qkn")
            nc.gpsimd.dma_start(qkn0[:, 0:S+kth0*128], qk_ap[s0, :, 0:S+kth0*128])
            vn0 = vpool.tile([128, NT * 132], BF16, name="vn", tag="vn")
            nc.gpsimd.dma_start(
                vn0.rearrange("p (t c) -> p t c", c=132)[:, 0:kth0, :],
                v_ap[s0, 0:kth0*128, :].rearrange("(t p) c -> p t c", p=128))
            prefetched[s0] = (qkn0, vn0)

        for w in range(WPC):
            ktw = int(slot_kt[w].max())
            # wmt[k=128, (kt, q)] tiles of sqrt(d)*wmask[w]^T for kt < ktw
            wmt = wmp.tile([128, NT * S], WMDT, name="wmt", tag="wmt")
            wmtv = wmt.rearrange("p (t q) -> p t q", q=S)
            nc.gpsimd.dma_start(
                wmtv[:, 0:ktw, :],
                wm_ap[w, 0:ktw*128, :].rearrange("(t p) q -> p t q", p=128))

            for i in range(HPW):
                s = w * HPW + i
                kth = int(slot_kt[w][i])

                if s in prefetched:
                    qkn, vn = prefetched[s]
                else:
                    qkn = qkp.tile([128, 2 * S], QKDT, name="qkn", tag="qkn")
                    nc.gpsimd.dma_start(qkn[:, 0:S+kth*128],
                                        qk_ap[s, :, 0:S+kth*128])
                    vn = vpool.tile([128, NT * 132], BF16, name="vn", tag="vn")
                    nc.gpsimd.dma_start(
                        vn.rearrange("p (t c) -> p t c", c=132)[:, 0:kth, :],
                        v_ap[s, 0:kth*128, :].rearrange("(t p) c -> p t c", p=128))
                vnv = vn.rearrange("p (t c) -> p t c", c=132)

                # k-tile pairs share a 2-bank PSUM tile so interior pairs can
                # run a single merged [128,1024] exp
                npair = (kth + 1) // 2
                pst = [ps.tile([128, 2 * S], F32, name="ps_t", tag="ps_t")
                       for _ in range(npair)]

                def ps_half(kt):
                    return pst[kt // 2][:, (kt % 2)*S:(kt % 2)*S + S]

                for kt in range(kth):
                    nc.tensor.matmul(ps_half(kt), ident[:], wmtv[:, kt, :],
                                     start=True, stop=False)

                pov = po.tile([128, NT * 256], F32, name="pov", tag="pov")
                ets = {}
                for kt in range(kth):
                    nc.tensor.matmul(ps_half(kt), qkn[:, S+kt*128:S+(kt+1)*128],
                                     qkn[:, 0:S], start=False, stop=True)
                    # exp as soon as a pair (or tail) is fully scored;
                    # valid-len masking rides the zeroed V'/ones rows, so
                    # scale/bias are the same immediates for every tile
                    if kt % 2 == 1 or kt == kth - 1:
                        p0 = kt - (kt % 2)
                        eT = etp.tile([128, 2 * S], BF16, name="eT", tag="eT")
                        width = (kt - p0 + 1) * S
                        nc.scalar.activation(
                            eT[:, 0:width], pst[kt // 2][:, 0:width],
                            EXP, bias=0.0, scale=ISD)
                        for k2 in range(p0, kt + 1):
                            ets[k2] = eT[:, (k2 % 2)*S:(k2 % 2)*S + S]
                            for qt in range(NT):
                                nc.tensor.matmul(
                                    pov[:, qt*256:qt*256+129],
                                    ets[k2][:, qt*128:(qt+1)*128],
                                    vnv[:, k2, 0:129],
                                    start=(k2 == 0 and qt % 2 == 0),
                                    stop=(k2 == kth-1 and qt % 2 == 1))

                povv = pov.rearrange("p (t c) -> p t c", c=256)
                ob = obp.tile([128, OBW], OBDT, name="ob", tag="ob")
                obv = ob.rearrange("p (t c) -> p t c", c=129)
                nc.vector.tensor_copy(obv[:], povv[:, :, 0:129])
                nc.sync.dma_start(o_ap[s], ob[:])
    nc.compile()
    return nc


def _make_in_maps(queries, keys, values, valid_lens, window_mask, hsel):
    import ml_dtypes
    qk_np_dt = np.float16 if QK16 else np.float32
    wm_np_dt = np.float16 if WM16 else np.float32
    sd = np.sqrt(np.float32(D))
    ident_np = np.eye(128, dtype=wm_np_dt)

    in_maps = []
    for c in range(N_CORES):
        hs = hsel[c]                                     # head for each slot
        lens = valid_lens[hs]
        kg = np.arange(S)
        valid = kg[None, :] < lens[:, None]              # [HPC(slots), S]

        qkn = np.empty((HPC, D, 2 * S), qk_np_dt)
        qkn[:, :, 0:S] = queries[hs].transpose(0, 2, 1)
        qkn[:, :, S:2*S] = keys[hs].transpose(0, 2, 1)

        # valid-length masking: rows k >= len contribute exactly 0 to both
        # the output accumulation and the softmax denominator
        vpk = np.zeros((HPC, S, 132), ml_dtypes.bfloat16)
        vpk[:, :, 0:128] = values[hs]
        vpk[:, :, 128] = 1.0
        vpk[~valid] = 0

        wmT = (sd * window_mask[4 * (c % 4): 4 * (c % 4) + 4]
               ).transpose(0, 2, 1)                      # [4, k, q]

        in_maps.append({
            "qk": np.ascontiguousarray(qkn),
            "vp": np.ascontiguousarray(vpk),
            "wmT": np.ascontiguousarray(wmT.astype(wm_np_dt)),
            "ident": ident_np,
        })
    return in_maps


def _postprocess(ob_core):
    """[HPC, 128, 516] device output -> [HPC, S, D] normalized, q-major."""
    a = np.asarray(ob_core, np.float32).reshape(HPC, 128, NT, 129)
    unnorm = a[:, :, :, 0:128]
    sums = a[:, :, :, 128:129]
    out = unnorm / sums                                  # [HPC, qp, qt, d]
    return out.transpose(0, 2, 1, 3).reshape(HPC, S, D)


def _install_ntff_hook():
    import types
    if "antenv.axon_hooks" in sys.modules:
        return
    try:
        from trn_agent_boot.trn_boot import _ntff_profile_via_ctypes
        hook = _ntff_profile_via_ctypes('/opt/axon/libaxon_pjrt.so')
    except Exception:
        hook = None
    mod = types.ModuleType("antenv.axon_hooks")
    mod.get_axon_ntff_profile_hook = lambda: hook
    mod.set_axon_ntff_profile_hook = lambda h: None
    sys.modules["antenv.axon_hooks"] = mod
    try:
        import antenv
        antenv.axon_hooks = mod
    except Exception:
        pass


_LAST_RESULTS = {}


def kernel(queries, keys, values, valid_lens, window_mask):
    queries = np.ascontiguousarray(np.asarray(queries, dtype=np.float32))
    keys = np.ascontiguousarray(np.asarray(keys, dtype=np.float32))
    values = np.ascontiguousarray(np.asarray(values, dtype=np.float32))
    valid_lens = np.asarray(valid_lens, dtype=np.int32)
    window_mask = np.ascontiguousarray(np.asarray(window_mask, dtype=np.float32))

    slot_kt, hsel = _plan(valid_lens)
    in_maps = _make_in_maps(queries, keys, values, valid_lens, window_mask, hsel)
    nc = _build_program(slot_kt)

    trace = os.environ.get("ATTN_TRACE", "0") == "1"
    if trace:
        _install_ntff_hook()
    res = run_bass_kernel_spmd(nc, in_maps, list(range(N_CORES)), trace=trace)
    _LAST_RESULTS["res"] = res

    out = np.empty((N, S, D), np.float32)
    for c in range(N_CORES):
        out[hsel[c]] = _postprocess(res.results[c]["o"])

    # len==0 heads: reference softmaxes an all-(-1e6) row -> uniform
    # attention -> mean of V; the device path can't represent that.
    for h in np.nonzero(valid_lens == 0)[0]:
        out[int(h)] = values[int(h)].mean(axis=0, keepdims=True)
    return out


# revision 14
# speedup vs baseline: 1.9746x; 1.2459x over previous
"""Trainium2 Bass kernel for nn_DotProductAttention_292057776923.

Per-head windowed attention with valid-length masking:
  out[h] = softmax(Q[h] K[h]^T / sqrt(d) + wmask[w(h)], masked k>=len[h]) @ V[h]
n=256 heads (B2 x W16 x H8), S=512, d=128, f32.

Sharding: pure head-parallel across 8 cores (32 consecutive heads each);
core c needs window masks [4*(c%4), 4*(c%4)+4). No cross-core communication.

v3 design (per head, scoresT layout [k, q] everywhere):
  - HOST pre-transposes Q, K -> one packed [d, 1024] fp16 slab per head,
    wmask -> sqrt(d)*wmask^T (fp16), and packs V' = [V | ones | pad] (bf16).
    No PE transposes, no PSUM->SBUF casts, no on-device mask prep.
  - inject:  ps[k,q]  = I.T @ wmT_tile          (PE matmul, start=True)
  - score:   ps[k,q] += KT_chunk.T @ QT          (PE, start=False)
  - exp:     eT[k,q]  = Exp(ps*scale[k]+bias[k]) (ACT -> bf16; k-tile PAIRS
             that are fully interior on every core run as one [128,1024]
             ACT with immediate scale/bias)
  - AV+sums: pov[q, qt*256 : qt*256+129] += eT_chunk.T @ [V|ones]   (bf16,
             N=129, accumulated over k-tiles in 2 PSUM banks)
  - ob = one DVE copy of the 4x129 regions -> SBUF (bf16); DMA out;
    HOST normalizes (out = unnorm / sums) and transposes back.
  - input DMA triggers ride the (otherwise idle) GpSimd queue; stores on
    Sync - avoids serializing ~100 DMA triggers on one engine.

Valid-length truncation: only ceil(len/128) k-tiles contribute. Heads are
sorted within each 8-head window group by needed tiles and the SPMD program
uses the per-slot max across cores. len==0 heads are overwritten on the host
with mean(V).
"""
import os
import sys

sys.path.insert(0, "/opt/trn_rl_repo")

import numpy as np
from contextlib import ExitStack

import concourse.bass as bass
import concourse.tile as tile
from concourse import bacc, mybir
from concourse.bass_utils import run_bass_kernel_spmd

F32 = mybir.dt.float32
F32R = mybir.dt.float32r
F16 = mybir.dt.float16
BF16 = mybir.dt.bfloat16
EXP = mybir.ActivationFunctionType.Exp

N, S, D = 256, 512, 128
NT = S // 128             # 4 k/q tiles per head
N_CORES = 8
HPC = N // N_CORES        # 32 heads per core
WPC = 4                   # window groups per core
HPW = HPC // WPC          # 8 heads per window group
MASK_BIAS = -60.0
OBW = NT * 129            # 516: per-q-tile [128 outs | 1 sum]
ISD = 1.0 / float(np.sqrt(np.float32(D)))

TRUNC = os.environ.get("ATTN_TRUNC", "1") == "1"
QK16 = os.environ.get("ATTN_QK16", "1") == "1"      # Q/K in fp16 (else f32r)
OUT16 = os.environ.get("ATTN_OUT16", "1") == "1"    # ob in bf16 (else f32)
WM16 = os.environ.get("ATTN_WM16", "1") == "1"      # wmT in fp16 (else f32r)
PAIR = os.environ.get("ATTN_PAIR", "1") == "1"      # merge interior exp pairs
QKDT = F16 if QK16 else F32R
OBDT = BF16 if OUT16 else F32
WMDT = F16 if WM16 else F32R


def _plan(valid_lens):
    """slot_kt[w][i]: k-tiles computed by slot i of window group w (uniform
    across cores); hsel[c][s]: GLOBAL head index assigned to core c, slot s.

    Cores c and c+4 use the same 4 windows, so each (window, core-pair)
    pool of 16 heads is sorted by k-tiles and snake-split: core c gets
    even ranks, c+4 odd ranks.  Slot i's k-tile count is then the rank-2i
    value, maxed over the 4 pools only."""
    kt_head = np.maximum(1, np.ceil(valid_lens / 128.0).astype(np.int64))
    if not TRUNC:
        kt_head[:] = NT
    hsel = np.empty((N_CORES, HPC), np.int64)
    slot_kt = np.zeros((WPC, HPW), np.int64)
    for p in range(4):                       # core pair (p, p+4)
        for w in range(WPC):
            gw = 4 * p + w                   # global window index
            pool = np.concatenate([
                np.arange(p * HPC + w * HPW, p * HPC + (w + 1) * HPW),
                np.arange((p + 4) * HPC + w * HPW, (p + 4) * HPC + (w + 1) * HPW),
            ])
            order = pool[np.argsort(-kt_head[pool], kind="stable")]
            hsel[p, w * HPW:(w + 1) * HPW] = order[0::2]
            hsel[p + 4, w * HPW:(w + 1) * HPW] = order[1::2]
            slot_kt[w] = np.maximum(slot_kt[w], kt_head[order[0::2]])
    return slot_kt, hsel


def _build_program(slot_kt):
    nc = bacc.Bacc("TRN2", target_bir_lowering=False, debug=False,
                   enable_asserts=True, num_devices=N_CORES)
    qk_ap = nc.dram_tensor("qk", [HPC, D, 2 * S], QKDT, kind="ExternalInput").ap()
    v_ap = nc.dram_tensor("vp", [HPC, S, 132], BF16, kind="ExternalInput").ap()
    wm_ap = nc.dram_tensor("wmT", [WPC, S, S], WMDT, kind="ExternalInput").ap()
    id_ap = nc.dram_tensor("ident", [128, 128], WMDT, kind="ExternalInput").ap()
    o_ap = nc.dram_tensor("o", [HPC, 128, OBW], OBDT, kind="ExternalOutput").ap()

    with tile.TileContext(nc) as tc, ExitStack() as ctx:
        const_p = ctx.enter_context(tc.tile_pool(name="const", bufs=1))
        qkp = ctx.enter_context(tc.tile_pool(name="qkp", bufs=4))
        vpool = ctx.enter_context(tc.tile_pool(name="vpool", bufs=4))
        wmp = ctx.enter_context(tc.tile_pool(name="wmp", bufs=2))
        etp = ctx.enter_context(tc.tile_pool(name="etp", bufs=3))
        obp = ctx.enter_context(tc.tile_pool(name="obp", bufs=3))
        ps = ctx.enter_context(tc.tile_pool(name="ps", bufs=3, space="PSUM"))
        po = ctx.enter_context(tc.tile_pool(name="po", bufs=1, space="PSUM"))

        ident = const_p.tile([128, 128], WMDT)
        nc.gpsimd.dma_start(ident[:], id_ap[:])

        # window-0 mask first, then prefetch three heads' q/k/v
        wmt0 = wmp.tile([128, NT * S], WMDT, name="wmt", tag="wmt")
        ktw0 = int(slot_kt[0].max())
        nc.gpsimd.dma_start(
            wmt0.rearrange("p (t q) -> p t q", q=S)[:, 0:ktw0, :],
            wm_ap[0, 0:ktw0*128, :].rearrange("(t p) q -> p t q", p=128))
        prefetched = {}
        for s0 in (0, 1, 2):
            kth0 = int(slot_kt[0][s0])
            qkn0 = qkp.tile([128, 2 * S], QKDT, name="qkn", tag="qkn")
            nc.gpsimd.dma_start(qkn0[:, 0:S+kth0*128], qk_ap[s0, :, 0:S+kth0*128])
            vn0 = vpool.tile([128, NT * 132], BF16, name="vn", tag="vn")
            nc.gpsimd.dma_start(
                vn0.rearrange("p (t c) -> p t c", c=132)[:, 0:kth0, :],
                v_ap[s0, 0:kth0*128, :].rearrange("(t p) c -> p t c", p=128))
            prefetched[s0] = (qkn0, vn0)

        for w in range(WPC):
            ktw = int(slot_kt[w].max())
            # wmt[k=128, (kt, q)] tiles of sqrt(d)*wmask[w]^T for kt < ktw
            if w == 0:
                wmt = wmt0
            else:
                wmt = wmp.tile([128, NT * S], WMDT, name="wmt", tag="wmt")
                nc.gpsimd.dma_start(
                    wmt.rearrange("p (t q) -> p t q", q=S)[:, 0:ktw, :],
                    wm_ap[w, 0:ktw*128, :].rearrange("(t p) q -> p t q", p=128))
            wmtv = wmt.rearrange("p (t q) -> p t q", q=S)

            for i in range(HPW):
                s = w * HPW + i
                kth = int(slot_kt[w][i])

                if s in prefetched:
                    qkn, vn = prefetched[s]
                else:
                    qkn = qkp.tile([128, 2 * S], QKDT, name="qkn", tag="qkn")
                    nc.gpsimd.dma_start(qkn[:, 0:S+kth*128],
                                        qk_ap[s, :, 0:S+kth*128])
                    vn = vpool.tile([128, NT * 132], BF16, name="vn", tag="vn")
                    nc.gpsimd.dma_start(
                        vn.rearrange("p (t c) -> p t c", c=132)[:, 0:kth, :],
                        v_ap[s, 0:kth*128, :].rearrange("(t p) c -> p t c", p=128))
                vnv = vn.rearrange("p (t c) -> p t c", c=132)

                # k-tile pairs share a 2-bank PSUM tile so interior pairs can
                # run a single merged [128,1024] exp
                npair = (kth + 1) // 2
                pst = [ps.tile([128, 2 * S], F32, name="ps_t", tag="ps_t")
                       for _ in range(npair)]

                def ps_half(kt):
                    return pst[kt // 2][:, (kt % 2)*S:(kt % 2)*S + S]

                for kt in range(kth):
                    nc.tensor.matmul(ps_half(kt), ident[:], wmtv[:, kt, :],
                                     start=True, stop=False)

                pov = po.tile([128, NT * 256], F32, name="pov", tag="pov")
                ets = {}
                for kt in range(kth):
                    nc.tensor.matmul(ps_half(kt), qkn[:, S+kt*128:S+(kt+1)*128],
                                     qkn[:, 0:S], start=False, stop=True)
                    # exp as soon as a pair (or tail) is fully scored;
                    # valid-len masking rides the zeroed V'/ones rows, so
                    # scale/bias are the same immediates for every tile
                    if kt % 2 == 1 or kt == kth - 1:
                        p0 = kt - (kt % 2)
                        eT = etp.tile([128, 2 * S], BF16, name="eT", tag="eT")
                        width = (kt - p0 + 1) * S
                        nc.scalar.activation(
                            eT[:, 0:width], pst[kt // 2][:, 0:width],
                            EXP, bias=0.0, scale=ISD)
                        for k2 in range(p0, kt + 1):
                            ets[k2] = eT[:, (k2 % 2)*S:(k2 % 2)*S + S]
                            for qt in range(NT):
                                nc.tensor.matmul(
                                    pov[:, qt*256:qt*256+129],
                                    ets[k2][:, qt*128:(qt+1)*128],
                                    vnv[:, k2, 0:129],
                                    start=(k2 == 0 and qt % 2 == 0),
                                    stop=(k2 == kth-1 and qt % 2 == 1))

                povv = pov.rearrange("p (t c) -> p t c", c=256)
                ob = obp.tile([128, OBW], OBDT, name="ob", tag="ob")
                obv = ob.rearrange("p (t c) -> p t c", c=129)
                nc.vector.tensor_copy(obv[:], povv[:, :, 0:129])
                nc.sync.dma_start(o_ap[s], ob[:])
    nc.compile()
    return nc


def _make_in_maps(queries, keys, values, valid_lens, window_mask, hsel):
    import ml_dtypes
    qk_np_dt = np.float16 if QK16 else np.float32
    wm_np_dt = np.float16 if WM16 else np.float32
    sd = np.sqrt(np.float32(D))
    ident_np = np.eye(128, dtype=wm_np_dt)

    in_maps = []
    for c in range(N_CORES):
        hs = hsel[c]                                     # head for each slot
        lens = valid_lens[hs]
        kg = np.arange(S)
        valid = kg[None, :] < lens[:, None]              # [HPC(slots), S]

        qkn = np.empty((HPC, D, 2 * S), qk_np_dt)
        qkn[:, :, 0:S] = queries[hs].transpose(0, 2, 1)
        qkn[:, :, S:2*S] = keys[hs].transpose(0, 2, 1)

        # valid-length masking: rows k >= len contribute exactly 0 to both
        # the output accumulation and the softmax denominator
        vpk = np.zeros((HPC, S, 132), ml_dtypes.bfloat16)
        vpk[:, :, 0:128] = values[hs]
        vpk[:, :, 128] = 1.0
        vpk[~valid] = 0

        wmT = (sd * window_mask[4 * (c % 4): 4 * (c % 4) + 4]
               ).transpose(0, 2, 1)                      # [4, k, q]

        in_maps.append({
            "qk": np.ascontiguousarray(qkn),
            "vp": np.ascontiguousarray(vpk),
            "wmT": np.ascontiguousarray(wmT.astype(wm_np_dt)),
            "ident": ident_np,
        })
    return in_maps


def _postprocess(ob_core):
    """[HPC, 128, 516] device output -> [HPC, S, D] normalized, q-major."""
    a = np.asarray(ob_core, np.float32).reshape(HPC, 128, NT, 129)
    unnorm = a[:, :, :, 0:128]
    sums = a[:, :, :, 128:129]
    out = unnorm / sums                                  # [HPC, qp, qt, d]
    return out.transpose(0, 2, 1, 3).reshape(HPC, S, D)


def _install_ntff_hook():
    import types
    if "antenv.axon_hooks" in sys.modules:
        return
    try:
        from trn_agent_boot.trn_boot import _ntff_profile_via_ctypes
        hook = _ntff_profile_via_ctypes('/opt/axon/libaxon_pjrt.so')
    except Exception:
        hook = None
    mod = types.ModuleType("antenv.axon_hooks")
    mod.get_axon_ntff_profile_hook = lambda: hook
    mod.set_axon_ntff_profile_hook = lambda h: None
    sys.modules["antenv.axon_hooks"] = mod
    try:
        import antenv
        antenv.axon_hooks = mod
    except Exception:
        pass


_LAST_RESULTS = {}


def kernel(queries, keys, values, valid_lens, window_mask):
    queries = np.ascontiguousarray(np.asarray(queries, dtype=np.float32))
    keys = np.ascontiguousarray(np.asarray(keys, dtype=np.float32))
    values = np.ascontiguousarray(np.asarray(values, dtype=np.float32))
    valid_lens = np.asarray(valid_lens, dtype=np.int32)
    window_mask = np.ascontiguousarray(np.asarray(window_mask, dtype=np.float32))

    slot_kt, hsel = _plan(valid_lens)
    in_maps = _make_in_maps(queries, keys, values, valid_lens, window_mask, hsel)
    nc = _build_program(slot_kt)

    trace = os.environ.get("ATTN_TRACE", "0") == "1"
    if trace:
        _install_ntff_hook()
    res = run_bass_kernel_spmd(nc, in_maps, list(range(N_CORES)), trace=trace)
    _LAST_RESULTS["res"] = res

    out = np.empty((N, S, D), np.float32)
    for c in range(N_CORES):
        out[hsel[c]] = _postprocess(res.results[c]["o"])

    # len==0 heads: reference softmaxes an all-(-1e6) row -> uniform
    # attention -> mean of V; the device path can't represent that.
    for h in np.nonzero(valid_lens == 0)[0]:
        out[int(h)] = values[int(h)].mean(axis=0, keepdims=True)
    return out
